# revision 2
# baseline (speedup 1.0000x reference)
"""Trainium2 Bass kernel: DiT block with cross-attention (nn_DiTBlock_CrossAttn).

Sharding: pure data-parallel over batch. B=8 -> 8 NeuronCores, no collectives.

v2 design vs baseline:
- Host feeds x/context/biases pre-transposed (feature-major); output is written
  feature-major and transposed back on host. No PE transposes of activations.
- All six attention-side projections (qkv, so, cq, ck, cv, co) run in fp8 e4m3
  with DoubleRow perf mode: one matmul contracts 256 rows (two 128-chunks), so
  instruction count and PE streaming time halve. Weights are host-scaled x32;
  the 1/32 dequant is folded into eviction scale/bias ops. FFN and attention
  internals stay bf16 (fp8 there breaks the 2e-2 accuracy gate).
- adaLN restructured: silu(c) chunks are the stationary operand (1-column
  ldweights) with w_ada streamed as the moving operand: 96 N=512 matmuls
  instead of 384 tiny ones.
- Attention processes head PAIRS: the two K=64 logit matmuls of heads 2i/2i+1
  target row-groups 0-63/64-127 and run concurrently on the PE array.
- FFN: gelu activations stay in SBUF (no DRAM roundtrip); w2 runs feature-major
  so the residual add needs no transposes.
- LayerNorm mean/E[x2] share one all-ones stationary operand; rstd is computed
  broadcast so no extra broadcast matmuls are needed.
- Residual stream in bf16.
"""
import contextlib

import numpy as np
import ml_dtypes

import concourse.bass as bass
import concourse.tile as tile
import concourse.mybir as mybir
from concourse import bacc
from concourse.bass_utils import run_bass_kernel_spmd
from concourse.masks import make_identity

P = 128
N = 1024            # tokens
D = 1024            # hidden
KD = D // P         # 8 feature chunks
NT = N // P         # 8 token tiles
H = 16              # heads
HD = 64             # head dim
S = 256             # context tokens
ST = S // P         # 2
CD = 512            # context dim
CKD = CD // P       # 4
MLP = 4096
MT = MLP // P       # 32
EPS = 1e-6
ASCALE = 0.125      # 1/sqrt(HD)
NCORES = 8
WS = 32.0           # fp8 weight pre-scale (host side)
IWS = 1.0 / WS  # overridden to 1.0 for bf16-proj variants at emit time

F32 = mybir.dt.float32
BF16 = mybir.dt.bfloat16
F8 = mybir.dt.float8e4
AF = mybir.ActivationFunctionType
OP = mybir.AluOpType
DR = mybir.MatmulPerfMode.DoubleRow


def _wcols(w):
    """[din, dout] dram AP -> [p, ko, dout] (feature-chunked lhsT view)."""
    return w.rearrange("(ko p) f -> p ko f", p=P)


def build_nc(taps=(), upto='full', proj_fp8=True):
    nc = bacc.Bacc("TRN2", target_bir_lowering=False, debug=False)

    PF8 = F8 if proj_fp8 else BF16
    d = {}
    d['xt'] = nc.dram_tensor("xt", [D, N], BF16, kind="ExternalInput").ap()
    d['ctx8'] = nc.dram_tensor("ctx8", [CD, S], PF8, kind="ExternalInput").ap()
    d['cT'] = nc.dram_tensor("cT", [P, KD], F32, kind="ExternalInput").ap()
    for nm, sh in [("w_qkv8", [D, 3 * D]), ("w_so8", [D, D]), ("w_cq8", [D, D]),
                   ("w_ck8", [CD, D]), ("w_cv8", [CD, D]), ("w_co8", [D, D])]:
        d[nm] = nc.dram_tensor(nm, sh, PF8, kind="ExternalInput").ap()
    for nm, sh in [("w1", [D, MLP]), ("w2", [MLP, D]), ("w_ada", [D, 6 * D])]:
        d[nm] = nc.dram_tensor(nm, sh, BF16, kind="ExternalInput").ap()
    for nm, w in [("bqkT", 16), ("bsoT", KD), ("bcqT", KD), ("bckT", KD),
                  ("bcoT", KD), ("b1T", MT), ("b2T", KD), ("badaT", 48)]:
        d[nm] = nc.dram_tensor(nm, [P, w], F32, kind="ExternalInput").ap()
    d['bv32'] = nc.dram_tensor("bv32", [D], F32, kind="ExternalInput").ap()
    d['bcv32'] = nc.dram_tensor("bcv32", [D], F32, kind="ExternalInput").ap()
    out = nc.dram_tensor("out_x", [D, N], F32, kind="ExternalOutput").ap()
    srows = nc.dram_tensor("srows", [32, N], F32).ap()

    tap_shapes = {
        "ada": ([P, 48], F32), "h1": ([P, KD, N], F8),
        "q": ([P, KD, N], BF16), "k": ([P, KD, N], BF16),
        "v65": ([P, NT, H, 65], BF16), "saO": ([P, KD, N], F8),
        "x2": ([P, KD, N], BF16), "h2": ([P, KD, N], F8),
        "cq": ([P, KD, N], BF16), "ck": ([P, KD, S], BF16),
        "cv65": ([P, ST, H, 65], BF16), "caO": ([P, KD, N], F8),
        "x3": ([P, KD, N], BF16), "h3": ([P, KD, N], BF16),
        "xT": ([P, KD, N], BF16), "g": ([P, MT, N], BF16),
    }
    tap_aps = {nm: nc.dram_tensor(f"dbg_{nm}", *tap_shapes[nm], kind="ExternalOutput").ap()
               for nm in taps}

    with tile.TileContext(nc) as tc:
        _emit(nc, tc, d, out, srows, tap_aps, upto, proj_fp8)
    nc.compile()
    return nc


def _emit(nc, tc, d, out, srows, tap_aps={}, upto='full', proj_fp8=True):
    def tap(nm, t):
        if nm in tap_aps:
            nc.sync.dma_start(tap_aps[nm], t[:])

    if not proj_fp8:
        iws_l, ws_l = 1.0, 1.0
    else:
        iws_l, ws_l = 1.0 / 32.0, 32.0
    PF8 = F8 if proj_fp8 else BF16
    KSTEP = 2 if proj_fp8 else 1
    PMODE = DR if proj_fp8 else None

    _ctr = [0]

    def nm(base):
        _ctr[0] += 1
        return f"{base}_{_ctr[0]}"

    gl = contextlib.ExitStack()
    with gl:
        const = gl.enter_context(tc.tile_pool(name="const", bufs=1))
        probe_p = gl.enter_context(tc.tile_pool(name="probe_p", bufs=1))

        def stage_out():
            pr = probe_p.tile([P, 512], F32, tag="probe")
            nc.vector.memset(pr[:], 1.0)
            nc.sync.dma_start(out[0:P, 0:512], pr[:])
        resid = gl.enter_context(tc.tile_pool(name="resid", bufs=2))
        actbf = gl.enter_context(tc.tile_pool(name="actbf", bufs=2))
        act8 = gl.enter_context(tc.tile_pool(name="act8", bufs=1))

        # ---------- constants ----------
        ident = const.tile([P, P], F32, tag="ident")
        make_identity(nc, ident)
        onesD_mat = const.tile([P, P], BF16, tag="onesD_mat")
        nc.vector.memset(onesD_mat[:], 1.0 / D)
        eps_t = const.tile([P, 1], F32, tag="eps")
        nc.vector.memset(eps_t[:], EPS)

        # pre-transposed biases straight from DRAM
        bt = {}
        for bn, w in [("bqkT", 16), ("bsoT", KD), ("bcqT", KD), ("bckT", KD),
                      ("bcoT", KD), ("b1T", MT), ("b2T", KD), ("badaT", 48)]:
            bt[bn] = const.tile([P, w], F32, tag=bn, name=bn)
            nc.sync.dma_start(bt[bn][:], d[bn])
        # x (feature-major, bf16) -- per-chunk DMAs so LN1 stats can start early
        xT = resid.tile([P, KD, N], BF16, tag="resid", name="xT")
        xt_cols = d['xt'].rearrange("(ko p) n -> p ko n", p=P)
        for k in range(KD):
            nc.sync.dma_start(xT[:, k, :], xt_cols[:, k, :])

        # silu(c) feature-major columns
        ctile = const.tile([P, KD], F32, tag="ctile")
        nc.sync.dma_start(ctile[:], d['cT'])
        silu_cT = const.tile([P, KD], BF16, tag="silu_cT")
        nc.scalar.activation(silu_cT[:], ctile[:], AF.Silu)

        ada = const.tile([P, 48], F32, tag="ada")
        splus = const.tile([P, 24], F32, tag="splus")

        # ---------- LayerNorm helpers ----------
        def ln_stats(x_in, stack):
            lnb = stack.enter_context(tc.tile_pool(name="lnb", bufs=3))
            ps_ln = stack.enter_context(tc.tile_pool(name="ps_ln", bufs=1, space="PSUM"))
            mu_ps = ps_ln.tile([P, N], F32, tag="mups")
            e2_ps = ps_ln.tile([P, N], F32, tag="e2ps")
            for k in range(KD):
                sq = lnb.tile([P, N], BF16, tag="lnsq", name=nm("lnsq"))
                nc.scalar.activation(sq[:], x_in[:, k], AF.Square)
                for half in range(2):
                    hs = slice(half * 512, (half + 1) * 512)
                    nc.tensor.matmul(mu_ps[:, hs], onesD_mat[:], x_in[:, k, hs],
                                     start=(k == 0), stop=(k == KD - 1))
                    nc.tensor.matmul(e2_ps[:, hs], onesD_mat[:], sq[:, hs],
                                     start=(k == 0), stop=(k == KD - 1))
            mu_bf = lnb.tile([P, N], BF16, tag="mubf")
            nc.vector.tensor_copy(mu_bf[:], mu_ps[:])
            rstd = lnb.tile([P, N], F32, tag="rstd")
            nc.vector.tensor_mul(rstd[:], mu_bf[:], mu_bf[:])
            nc.vector.tensor_sub(rstd[:], e2_ps[:], rstd[:])
            nc.scalar.activation(rstd[:], rstd[:], AF.Sqrt, bias=eps_t[:])
            nc.vector.reciprocal(rstd[:], rstd[:])
            rstd_bf = lnb.tile([P, N], BF16, tag="rstdbf")
            nc.vector.tensor_copy(rstd_bf[:], rstd[:])
            return mu_bf, rstd_bf

        def ln_apply(x_in, g, mu_bf, rstd_bf, h_out, tpool):
            # modulate alternates Pool/ACT so neither serializes the next stage
            for k in range(KD):
                t1 = tpool.tile([P, N], BF16, tag="t1", name=nm("t1"))
                nc.vector.tensor_sub(t1[:], x_in[:, k], mu_bf[:])
                nc.vector.tensor_mul(t1[:], t1[:], rstd_bf[:])
                sp = splus[:, g * 8 + k:g * 8 + k + 1]
                sh = ada[:, g * 16 + k:g * 16 + k + 1]
                if k % 2 == 0:
                    nc.gpsimd.tensor_scalar(h_out[:, k], t1[:], sp, sh,
                                            OP.mult, OP.add)
                else:
                    nc.scalar.activation(h_out[:, k], t1[:], AF.Identity,
                                         bias=sh, scale=sp)

        def ln_mod(x_in, g, h_out):
            ls = contextlib.ExitStack()
            with ls:
                mu_ps, rstd = ln_stats(x_in, ls)
                tp = ls.enter_context(tc.tile_pool(name="lnt", bufs=2))
                ln_apply(x_in, g, mu_ps, rstd, h_out, tp)

        # ---------- fp8 DoubleRow projections ----------
        # one [P, n_free] PSUM tile (2 banks for n_free=1024) per t8 block;
        # matmuls fill 512-wide halves, eviction runs once on the full width
        def proj8(ps_pool, wp, w8cols, kdin, a8, n_free, dout, evict,
                  wdt=None, kstep=None, pmode='dflt'):
            wdt = PF8 if wdt is None else wdt
            kstep = KSTEP if kstep is None else kstep
            pmode = PMODE if pmode == 'dflt' else pmode
            nhalf = max(1, n_free // 512)
            nw = min(512, n_free)
            kp_n = kdin // kstep
            for blk in range(dout // 1024):
                wb = wp.tile([P, kdin, 1024], wdt, tag="w8", name=nm("w8"))
                nc.sync.dma_start(wb[:], w8cols[:, :, blk * 1024:(blk + 1) * 1024])
                for t8 in range(8):
                    ps = ps_pool.tile([P, nhalf * nw], F32, tag="pmm", name=nm("pmm"))
                    for kp in range(kp_n):
                        ks = slice(kstep * kp, kstep * kp + kstep)
                        for half in range(nhalf):
                            hs = slice(half * 512, half * 512 + nw)
                            nc.tensor.matmul(ps[:, half * nw:half * nw + nw],
                                             wb[:, ks, t8 * 128:(t8 + 1) * 128]
                                             if kstep > 1 else
                                             wb[:, ks.start, t8 * 128:(t8 + 1) * 128],
                                             a8[:, ks, hs] if kstep > 1
                                             else a8[:, ks.start, hs],
                                             start=(kp == 0), stop=(kp == kp_n - 1),
                                             perf_mode=pmode)
                    evict(blk * 8 + t8, ps)

        def proj_V8(ps_pool, wp, w8cols, kdin, a8, m_tiles, v65t, bias_b):
            kp_n = kdin // KSTEP
            wb = wp.tile([P, kdin, 1024], PF8, tag="w8", name=nm("w8v"))
            nc.sync.dma_start(wb[:], w8cols[:])
            for i in range(m_tiles):
                ps0 = ps_pool.tile([P, 512], F32, tag="pmm", name=nm("pv0"))
                ps1 = ps_pool.tile([P, 512], F32, tag="pmm", name=nm("pv1"))
                for kp in range(kp_n):
                    ks = slice(KSTEP * kp, KSTEP * kp + KSTEP)
                    a_sl = a8[:, ks, i * 128:(i + 1) * 128]
                    nc.tensor.matmul(ps0[:], a_sl, wb[:, ks, 0:512],
                                     start=(kp == 0), stop=(kp == kp_n - 1), perf_mode=PMODE)
                    nc.tensor.matmul(ps1[:], a_sl, wb[:, ks, 512:1024],
                                     start=(kp == 0), stop=(kp == kp_n - 1), perf_mode=PMODE)
                for blk, ps in ((0, ps0), (1, ps1)):
                    nc.vector.tensor_add(
                        v65t[:, i, blk * 8:(blk + 1) * 8, 0:64],
                        ps.rearrange("p (h e) -> p h e", h=8),
                        bias_b[:, blk * 512:(blk + 1) * 512]
                        .rearrange("p (h e) -> p h e", h=8))
            nc.vector.memset(v65t[:, :, :, 64:65], ws_l)

        # ---------- attention core (bf16, head pairs) ----------
        # logits for one (head, key-tile) land in a [P, N] PSUM tile (both
        # token halves) so exp runs once per tile; softmax normalize reads
        # the pv accumulator straight from PSUM (reciprocal -> Pool
        # partition-broadcast -> multiply), no staging copy.
        # stage_copy: evict pv to SBUF before normalizing (frees the PSUM
        # slot one op sooner; right when DVE has headroom, i.e. self-attn)
        def attention(q_T, kv_T, v65t, m_tiles, o8, srow_base, stage_copy=True):
            at = contextlib.ExitStack()
            with at:
                expp = at.enter_context(tc.tile_pool(name="expp", bufs=4))
                arows = at.enter_context(tc.tile_pool(name="arows", bufs=2))
                rb = at.enter_context(tc.tile_pool(name="rb", bufs=2))
                ps_lg = at.enter_context(tc.tile_pool(name="ps_lg", bufs=2, space="PSUM"))
                ps_pv = at.enter_context(tc.tile_pool(name="ps_pv", bufs=2, space="PSUM"))
                m_pairs = m_tiles // 2
                for hp in range(8):
                    ha, hb = 2 * hp, 2 * hp + 1
                    pv_a = ps_pv.tile([65, N], F32, tag="pv", name=f"pva{srow_base}_{hp}")
                    pv_b = ps_pv.tile([65, N], F32, tag="pv", name=f"pvb{srow_base}_{hp}")
                    # ex tiles hold a key-tile PAIR [P, 2, N] so the PV matmul
                    # can contract both via fp8 DoubleRow
                    exs = [None] * m_pairs

                    def lgexp(mt):
                        if mt % 2 == 0:
                            exs[mt // 2] = [
                                expp.tile([P, 2, N], PF8, tag="ex",
                                          name=f"ex{srow_base}_{hp}_{mt}_{idx}")
                                for idx in range(2)]
                        for idx, off in ((0, 0), (1, 64)):
                            lg = ps_lg.tile([P, N], F32, tag="lg",
                                            name=f"lg{srow_base}_{hp}_{mt}_{idx}")
                            for half in range(2):
                                hs = slice(half * 512, (half + 1) * 512)
                                nc.tensor.matmul(lg[:, hs],
                                                 kv_T[off:off + 64, hp, mt * 128:(mt + 1) * 128],
                                                 q_T[off:off + 64, hp, hs],
                                                 start=True, stop=True)
                            nc.scalar.activation(exs[mt // 2][idx][:, mt % 2, :],
                                                 lg[:], AF.Exp, scale=ASCALE)

                    def pvacc(p):
                        for half in range(2):
                            hs = slice(half * 512, (half + 1) * 512)
                            nc.tensor.matmul(pv_a[:, hs],
                                             v65t[:, 2 * p:2 * p + 2, ha, :],
                                             exs[p][0][:, :, hs],
                                             start=(p == 0), stop=(p == m_pairs - 1),
                                             perf_mode=PMODE)
                            nc.tensor.matmul(pv_b[:, hs],
                                             v65t[:, 2 * p:2 * p + 2, hb, :],
                                             exs[p][1][:, :, hs],
                                             start=(p == 0), stop=(p == m_pairs - 1),
                                             perf_mode=PMODE)

                    for i in range(m_tiles + 2):
                        if i < m_tiles:
                            lgexp(i)
                        if i >= 2 and i % 2 == 1:
                            pvacc((i - 2) // 2)
                    for idx, (pv, h) in enumerate(((pv_a, ha), (pv_b, hb))):
                        if stage_copy:
                            pvs = rb.tile([65, N], F32, tag="pvs", name=nm("pvs"))
                            nc.vector.tensor_copy(pvs[:], pv[:])
                            pv = pvs
                        rec = arows.tile([1, N], F32, tag="row", name=nm("rec"))
                        nc.vector.reciprocal(rec[:], pv[64:65, :])
                        rbt = rb.tile([64, N], F32, tag="rbt", name=nm("rbt"))
                        nc.gpsimd.partition_broadcast(rbt[:], rec[:], 64)
                        off = idx * 64
                        nc.vector.tensor_mul(o8[off:off + 64, hp, :], pv[0:64, :], rbt[:])

        # ================= phase 0: LN1 stats + ada =================
        ln1 = contextlib.ExitStack()
        mu1, rstd1 = ln_stats(xT, ln1)

        ada_es = contextlib.ExitStack()
        with ada_es:
            adap = ada_es.enter_context(tc.tile_pool(name="adap", bufs=2))
            wadap = ada_es.enter_context(tc.tile_pool(name="wadap", bufs=2))
            ps_ada = ada_es.enter_context(tc.tile_pool(name="ps_ada", bufs=2, space="PSUM"))
            ps_tr = ada_es.enter_context(tc.tile_pool(name="ps_tr", bufs=1, space="PSUM"))
            adarow = adap.tile([12, 512], F32, tag="adarow")
            wada_cols = _wcols(d['w_ada'])
            for blk in range(6):
                wb = wadap.tile([P, KD, 1024], BF16, tag="wada", name=nm("wada"))
                nc.sync.dma_start(wb[:], wada_cols[:, :, blk * 1024:(blk + 1) * 1024])
                for tb in range(2):
                    ps = ps_ada.tile([1, 512], F32, tag="psada", name=nm("psada"))
                    for k in range(KD):
                        nc.tensor.matmul(ps[:], silu_cT[:, k:k + 1],
                                         wb[:, k, tb * 512:(tb + 1) * 512],
                                         start=(k == 0), stop=(k == KD - 1))
                    r = blk * 2 + tb
                    ast = adap.tile([1, 512], F32, tag="ast", name=nm("ast"))
                    nc.scalar.activation(ast[:], ps[:], AF.Copy)
                    nc.sync.dma_start(adarow[r:r + 1, :], ast[:])
            # transpose adarow [12,512] -> ada [P,48]
            tp = ps_tr.tile([P, 512], F32, tag="ptr")
            for j in range(4):
                nc.tensor.transpose(tp[:, j * 12:(j + 1) * 12],
                                    adarow[:, j * 128:(j + 1) * 128], ident[0:12, 0:12])
            ada4 = ada.rearrange("p (r j) -> p r j", j=4)
            for j in range(4):
                nc.vector.tensor_copy(ada4[:, :, j], tp[:, j * 12:(j + 1) * 12])
            nc.vector.tensor_add(ada[:], ada[:], bt['badaT'][:])
            for g in range(3):
                nc.vector.tensor_scalar_add(splus[:, g * 8:(g + 1) * 8],
                                            ada[:, g * 16 + 8:g * 16 + 16], 1.0)
        tap("ada", ada)
        tap("xT", xT)

        # deferred non-critical loads (keep HBM clear for w_ada up front)
        ctx8 = const.tile([P, CKD, S], PF8, tag="ctx8")
        nc.sync.dma_start(ctx8[:], d['ctx8'].rearrange("(ko p) n -> p ko n", p=P))
        vbias = const.tile([P, D], F32, tag="vbias")
        nc.sync.dma_start(vbias[:], d['bv32'][None, :].partition_broadcast(P))
        cvbias = const.tile([P, D], F32, tag="cvbias")
        nc.sync.dma_start(cvbias[:], d['bcv32'][None, :].partition_broadcast(P))

        # ================= LN1 apply -> h1 (fp8) =================
        h1 = act8.tile([P, KD, N], PF8, tag="a8", name="h1")
        lnt1 = contextlib.ExitStack()
        with lnt1:
            tp1 = lnt1.enter_context(tc.tile_pool(name="lnt", bufs=2))
            ln_apply(xT, 0, mu1, rstd1, h1, tp1)
        ln1.close()
        tap("h1", h1)
        if upto == 'ada':
            stage_out()
            return

        # fold so-bias into the residual (xT last read by ln_apply above)
        for k in range(KD):
            nc.vector.tensor_scalar_add(xT[:, k, :], xT[:, k, :], bt['bsoT'][:, k:k + 1])

        # ================= self-attention =================
        sa_es = contextlib.ExitStack()
        vp = sa_es.enter_context(tc.tile_pool(name="vp", bufs=1))
        v65 = vp.tile([P, NT, H, 65], PF8, tag="v65")

        qkv_ps = contextlib.ExitStack()
        ps_mm = qkv_ps.enter_context(tc.tile_pool(name="ps_mm", bufs=4, space="PSUM"))
        wq_p = qkv_ps.enter_context(tc.tile_pool(name="wq_p", bufs=2))
        wq_cols = _wcols(d['w_qkv8'])
        proj_V8(ps_mm, wq_p, wq_cols[:, :, 2 * D:3 * D], KD, h1, NT, v65, vbias)

        qT = actbf.tile([P, KD, N], BF16, tag="abf", name="qT")
        kT = actbf.tile([P, KD, N], BF16, tag="abf", name="kT")

        def ev_qk(t, ps):
            # ACT is idle during the qkv phase (exp hasn't started)
            dst = qT if t < 8 else kT
            nc.scalar.activation(dst[:, t % 8, :], ps[:], AF.Identity,
                                 bias=bt['bqkT'][:, t:t + 1], scale=iws_l)
        proj8(ps_mm, wq_p, wq_cols[:, :, 0:2 * D], KD, h1, N, 2 * D, ev_qk)
        tap("q", qT); tap("k", kT); tap("v65", v65)
        qkv_ps.close()
        if upto == 'qkv':
            sa_es.close()
            stage_out()
            return

        saO = act8.tile([P, KD, N], PF8, tag="a8", name="saO")
        attention(qT, kT, v65, NT, saO, 0)
        tap("saO", saO)
        sa_es.close()
        if upto == 'sa':
            stage_out()
            return

        x2T = resid.tile([P, KD, N], BF16, tag="resid", name="x2T")
        so_ps = contextlib.ExitStack()
        ps_mm = so_ps.enter_context(tc.tile_pool(name="ps_mm", bufs=4, space="PSUM"))
        wso_p = so_ps.enter_context(tc.tile_pool(name="wso_p", bufs=1))

        def ev_so(t, ps):
            nc.vector.scalar_tensor_tensor(x2T[:, t, :], ps[:], iws_l, xT[:, t, :],
                                           OP.mult, OP.add)
        proj8(ps_mm, wso_p, _wcols(d['w_so8']), KD, saO, N, D, ev_so)
        tap("x2", x2T)
        so_ps.close()
        if upto == 'so':
            stage_out()
            return

        # ================= cross-attention =================
        ca_es = contextlib.ExitStack()
        kp_ = ca_es.enter_context(tc.tile_pool(name="kp", bufs=1))
        vp2 = ca_es.enter_context(tc.tile_pool(name="vp2", bufs=1))
        ckT = kp_.tile([P, KD, S], BF16, tag="ckT")
        cv65 = vp2.tile([P, ST, H, 65], PF8, tag="cv65")

        ckcv_ps = contextlib.ExitStack()
        ps_kv = ckcv_ps.enter_context(tc.tile_pool(name="ps_kv", bufs=2, space="PSUM"))
        wkv_p = ckcv_ps.enter_context(tc.tile_pool(name="wkv_p", bufs=2))

        def ev_ck(t, ps):
            nc.vector.tensor_scalar(ckT[:, t, :], ps[:, 0:S],
                                    bt['bckT'][:, t:t + 1], iws_l,
                                    OP.add, OP.mult)
        proj8(ps_kv, wkv_p, _wcols(d['w_ck8']), CKD, ctx8, S, D, ev_ck)
        proj_V8(ps_kv, wkv_p, _wcols(d['w_cv8']), CKD, ctx8, ST, cv65, cvbias)
        tap("ck", ckT); tap("cv65", cv65)

        h2 = act8.tile([P, KD, N], PF8, tag="a8", name="h2")
        ln_mod(x2T, 1, h2)
        ckcv_ps.close()
        tap("h2", h2)

        # fold co-bias into x2T (last read by ln_mod above)
        for k in range(KD):
            nc.vector.tensor_scalar_add(x2T[:, k, :], x2T[:, k, :], bt['bcoT'][:, k:k + 1])

        cqT = actbf.tile([P, KD, N], BF16, tag="abf", name="cqT")
        ca_ps = contextlib.ExitStack()
        ps_mm = ca_ps.enter_context(tc.tile_pool(name="ps_mm", bufs=4, space="PSUM"))
        wcq_p = ca_ps.enter_context(tc.tile_pool(name="wcq_p", bufs=1))

        def ev_cq(t, ps):
            nc.vector.tensor_scalar(cqT[:, t, :], ps[:],
                                    bt['bcqT'][:, t:t + 1], iws_l,
                                    OP.add, OP.mult)
        proj8(ps_mm, wcq_p, _wcols(d['w_cq8']), KD, h2, N, D, ev_cq)
        tap("cq", cqT)
        ca_ps.close()
        if upto == 'cq':
            ca_es.close()
            stage_out()
            return

        caO = act8.tile([P, KD, N], PF8, tag="a8", name="caO")
        attention(cqT, ckT, cv65, ST, caO, 16, stage_copy=False)
        tap("caO", caO)
        ca_es.close()

        x3T = resid.tile([P, KD, N], BF16, tag="resid", name="x3T")
        co_ps = contextlib.ExitStack()
        ps_mm = co_ps.enter_context(tc.tile_pool(name="ps_mm", bufs=4, space="PSUM"))
        wco_p = co_ps.enter_context(tc.tile_pool(name="wco_p", bufs=1))

        def ev_co(t, ps):
            nc.vector.scalar_tensor_tensor(x3T[:, t, :], ps[:], iws_l, x2T[:, t, :],
                                           OP.mult, OP.add)
        proj8(ps_mm, wco_p, _wcols(d['w_co8']), KD, caO, N, D, ev_co)
        tap("x3", x3T)
        co_ps.close()
        if upto == 'ca':
            stage_out()
            return

        # ================= FFN (bf16: fp8 breaks the 2e-2 gate) =================
        h3 = actbf.tile([P, KD, N], BF16, tag="abf", name="h3")
        ln_mod(x3T, 2, h3)
        tap("h3", h3)
        # fold b2 into the residual before the final eviction
        for k in range(KD):
            nc.vector.tensor_scalar_add(x3T[:, k, :], x3T[:, k, :], bt['b2T'][:, k:k + 1])

        ffn_es = contextlib.ExitStack()
        gp = ffn_es.enter_context(tc.tile_pool(name="gp", bufs=1))
        g = gp.tile([P, MT, N], BF16, tag="g")

        w1_es = contextlib.ExitStack()
        ps_mm = w1_es.enter_context(tc.tile_pool(name="ps_mm", bufs=4, space="PSUM"))
        w1_p = w1_es.enter_context(tc.tile_pool(name="w1_p", bufs=2))

        def ev_gelu(t, ps):
            nc.scalar.activation(g[:, t, :], ps[:], AF.Gelu,
                                 bias=bt['b1T'][:, t:t + 1])
        proj8(ps_mm, w1_p, _wcols(d['w1']), KD, h3, N, MLP, ev_gelu,
              wdt=BF16, kstep=1, pmode=None)
        tap("g", g)
        w1_es.close()
        if upto == 'w1':
            ffn_es.close()
            stage_out()
            return

        w2_es = contextlib.ExitStack()
        outst = w2_es.enter_context(tc.tile_pool(name="outst", bufs=4))
        ps_po = w2_es.enter_context(tc.tile_pool(name="ps_po", bufs=4, space="PSUM"))
        w2_p = w2_es.enter_context(tc.tile_pool(name="w2_p", bufs=2))
        w2_cols = d['w2'].rearrange("(ko p) f -> p ko f", p=P)
        for t8 in range(8):
            wb = w2_p.tile([P, MT, P], BF16, tag="w2b", name=nm("w2b"))
            nc.sync.dma_start(wb[:], w2_cols[:, :, t8 * 128:(t8 + 1) * 128])
            pso = ps_po.tile([P, N], F32, tag="po", name=nm("po"))
            for m in range(MT):
                nc.tensor.matmul(pso[:, 0:512], wb[:, m, :], g[:, m, 0:512],
                                 start=(m == 0), stop=(m == MT - 1))
                nc.tensor.matmul(pso[:, 512:1024], wb[:, m, :], g[:, m, 512:1024],
                                 start=(m == 0), stop=(m == MT - 1))
            ost = outst.tile([P, N], F32, tag="ost", name=nm("ost"))
            nc.vector.tensor_add(ost[:], pso[:], x3T[:, t8, :])
            nc.sync.dma_start(out[t8 * 128:(t8 + 1) * 128, :], ost[:])
        w2_es.close()
        ffn_es.close()


_NC = None


def _get_nc():
    global _NC
    if _NC is None:
        _NC = build_nc()
    return _NC


def make_in_maps(inputs, proj_fp8=True):
    f8 = ml_dtypes.float8_e4m3 if proj_fp8 else ml_dtypes.bfloat16
    ws = WS if proj_fp8 else 1.0
    bf = ml_dtypes.bfloat16
    f32 = np.float32
    shared = {}
    for src, dst in [("w_qkv", "w_qkv8"), ("w_so", "w_so8"), ("w_cq", "w_cq8"),
                     ("w_ck", "w_ck8"), ("w_cv", "w_cv8"), ("w_co", "w_co8")]:
        shared[dst] = np.ascontiguousarray(
            (np.asarray(inputs[src], f32) * ws).astype(f8))
    for nm in ("w1", "w2", "w_ada"):
        shared[nm] = np.ascontiguousarray(np.asarray(inputs[nm]).astype(bf))
    bq = np.asarray(inputs['b_qkv'], f32)
    # bcqT/bckT are prescaled by ws (DVE evicts (ps + b*ws) * iws);
    # bqkT is not (ACT evicts identity(ps*iws + b))
    shared['bqkT'] = np.ascontiguousarray(bq[:2 * D].reshape(16, P).T)
    shared['bv32'] = np.ascontiguousarray(ws * bq[2 * D:])
    shared['bcv32'] = np.ascontiguousarray(ws * np.asarray(inputs['b_cv'], f32))
    for src, dst, w, sc in [("b_so", "bsoT", KD, 1.0), ("b_cq", "bcqT", KD, ws),
                            ("b_ck", "bckT", KD, ws), ("b_co", "bcoT", KD, 1.0),
                            ("b1", "b1T", MT, 1.0), ("b2", "b2T", KD, 1.0),
                            ("b_ada", "badaT", 48, 1.0)]:
        shared[dst] = np.ascontiguousarray(
            sc * np.asarray(inputs[src], f32).reshape(w, P).T)
    x = np.asarray(inputs['x'], f32)
    c = np.asarray(inputs['c'], f32)
    ctxt = np.asarray(inputs['context'], f32)
    in_maps = []
    for i in range(NCORES):
        m = dict(shared)
        m['xt'] = np.ascontiguousarray(x[i].T.astype(bf))
        m['ctx8'] = np.ascontiguousarray(ctxt[i].T.astype(f8))
        m['cT'] = np.ascontiguousarray(c[i].reshape(KD, P).T)
        in_maps.append(m)
    return in_maps


def kernel(**inputs):
    nc = _get_nc()
    in_maps = make_in_maps(inputs)
    res = run_bass_kernel_spmd(nc, in_maps, core_ids=list(range(NCORES)))
    return np.stack([res.results[i]["out_x"].T for i in range(NCORES)]).astype(np.float32)


if __name__ == "__main__":
    data = np.load("/root/problem/inputs.npz")
    out = kernel(**{k: data[k] for k in data.files})
    gold = np.load("/root/problem/gold64.npy")
    err = np.abs(out - gold)
    print("max abs err:", err.max(), " rel:", err.max() / np.abs(gold).max())

